# revision 1
# baseline (speedup 1.0000x reference)
"""AAM-Softmax (ArcFace) logits kernel for Trainium2, 8 NeuronCores.

Math (per reference):
    cosine = l2norm(input) @ l2norm(weight).T            # [B, C]
    tgt    = cosine[i, label[i]]
    phi    = tgt*cos(m) - sqrt(1-tgt^2)*sin(m)
    out    = S * cosine, except out[i, label[i]] = S * where(tgt>0, phi, tgt)

Sharding: weight/cosine column-sharded over 8 cores (vocab parallel);
input + labels replicated.  Core k owns classes [k*CS, (k+1)*CS).

Per-core device pipeline:
  - x [B, D] f32 -> row sumsq -> xinvS = S/||x|| (and xinv = 1/||x||)
  - xhatS = x * xinvS (bf16), PE-transposed into xT [D, B] bf16
  - wt input is the host-relayouted W.T shard [2, 128, CS] in bf16 (the
    matmul consumes bf16 anyway; shipping bf16 halves the weight DMA).
    Per 500-col tile: square (DVE) and ones-matmul -> column sumsq
    broadcast over partitions in PSUM; ACT sqrt + fast reciprocal ->
    winv tile [128, 500], folded into the weights (gpsimd).
  - main matmul: out_psum[b-tile] = xT.T @ wt_bf (K=256 over 2 chunks)
  - staging = out_psum * winv  (fuses the weight-norm column scale; x side
    already carries S), DMA to out[b-tile, c-tile].
  - margin: w_sel = weight[label] (host gather, replicated input; all
    arithmetic on device): tgt = (x . wsel) * xinv * wselinv; phi/select
    math on [128, 8]; final values scattered into out[i, label_local[i]]
    via indirect DMA (out-of-shard rows get OOB offsets and are skipped).
"""

import sys

if "/opt/trn_rl_repo" not in sys.path:
    sys.path.insert(0, "/opt/trn_rl_repo")

from dataclasses import dataclass

import ml_dtypes
import numpy as np

S = 50.0
MARGIN = 0.5
COS_M = float(np.cos(MARGIN))
SIN_M = float(np.sin(MARGIN))
OOB = 16000000.0  # exact in f32, > any valid flat offset


@dataclass(frozen=True)
class Cfg:
    b: int = 1024
    d: int = 256
    c: int = 100000
    ncores: int = 8
    tc: int = 500

    @property
    def cs(self):
        return self.c // self.ncores

    @property
    def nb(self):
        return self.b // 128

    @property
    def nkt(self):
        return self.d // 128

    @property
    def nct(self):
        return self.cs // self.tc

    @property
    def ncg(self):
        return min(5, self.nct)


def build(cfg: Cfg):
    import concourse.bass as bass
    import concourse.tile as tile
    from concourse import bacc, mybir
    from concourse.masks import make_identity

    f32 = mybir.dt.float32
    bf16 = mybir.dt.bfloat16
    i32 = mybir.dt.int32
    X = mybir.AxisListType.X
    Op = mybir.AluOpType
    Act = mybir.ActivationFunctionType

    b, d, cs, tc = cfg.b, cfg.d, cfg.cs, cfg.tc
    nb, nkt, nct = cfg.nb, cfg.nkt, cfg.nct

    nc = bacc.Bacc(
        "TRN2", target_bir_lowering=False, debug=False, num_devices=cfg.ncores
    )

    x_ext = nc.dram_tensor("x", [b, d], f32, kind="ExternalInput")
    wt_ext = nc.dram_tensor("wt", [nkt, 128, cs], bf16, kind="ExternalInput")
    wsel_ext = nc.dram_tensor("wsel", [b, d], f32, kind="ExternalInput")
    labrel_ext = nc.dram_tensor("labrel", [128, nb], i32, kind="ExternalInput")
    # group-major layout: each staging DMA lands fully contiguous in HBM;
    # the host de-interleaves groups on assembly
    out_blocks = [
        nc.dram_tensor(
            f"out{bi}",
            [cs // (cfg.ncg * cfg.tc), 128, cfg.ncg * cfg.tc],
            f32,
            kind="ExternalOutput",
        )
        for bi in range(b // 128)
    ]

    # c-tiles are processed in groups; each (b-tile, group) accumulates a
    # wide staging tile so the out DMA moves ncg*tc*4 bytes per partition row
    ncg = cfg.ncg  # c-tiles per group
    assert nct % ncg == 0
    with tile.TileContext(nc) as tc_:
        with (
            tc_.tile_pool(name="const", bufs=1) as constp,
            tc_.tile_pool(name="persist", bufs=1) as persist,
            tc_.tile_pool(name="xin", bufs=2) as xin,
            tc_.tile_pool(name="xsc", bufs=2) as xsc,
            tc_.tile_pool(name="tiny", bufs=2) as tiny,
            tc_.tile_pool(name="wstream", bufs=4 * ncg) as wstream,
            tc_.tile_pool(name="wbf", bufs=2 * 2 * ncg) as wbf,
            tc_.tile_pool(name="winvp", bufs=ncg + 2) as winvp,
            tc_.tile_pool(name="stage", bufs=6) as stage,
            tc_.tile_pool(name="pn", bufs=2, space="PSUM") as pn,
            tc_.tile_pool(name="po", bufs=ncg + 1, space="PSUM") as po,
        ):
            ident_bf = constp.tile([128, 128], bf16)
            make_identity(nc, ident_bf[:])
            ones_bf = constp.tile([128, 128], bf16)
            nc.vector.memset(ones_bf[:], 1.0)

            # persistent tensors
            xT = persist.tile([128, nkt * b], bf16)  # [d-half on part][k*b + i]
            labrel_t = persist.tile([128, nb], i32)
            rel_f = persist.tile([128, nb], f32)
            iota_i = persist.tile([128, nb], i32)
            iota_f = persist.tile([128, nb], f32)
            xinv8 = persist.tile([128, nb], f32)
            wsinv8 = persist.tile([128, nb], f32)
            rawdot8 = persist.tile([128, nb], f32)
            newv8 = persist.tile([128, nb], f32)
            offs_i = persist.tile([128, nb], i32)

            nc.sync.dma_start(labrel_t[:], labrel_ext[:])
            # per-block flat offset base = p*(ncg*tc); the group term and
            # OOB sentinel are host-encoded into labrel
            nc.gpsimd.iota(
                iota_i[:], pattern=[[0, nb]], base=0,
                channel_multiplier=ncg * tc,
            )
            nc.vector.tensor_copy(iota_f[:], iota_i[:])
            nc.vector.tensor_copy(rel_f[:], labrel_t[:])

            # ---- Phase A: x prep ----
            ss8 = persist.tile([128, nb], f32)
            wss8 = persist.tile([128, nb], f32)
            x_tiles = []
            for bi in range(nb):
                rsl = slice(bi * 128, (bi + 1) * 128)
                x_t = xin.tile([128, d], f32, tag="x_t", name="x_t", bufs=nb)
                nc.sync.dma_start(x_t[:], x_ext[rsl, :])
                x_tiles.append(x_t)
                sq = xsc.tile([128, d], f32)
                nc.vector.tensor_mul(sq[:], x_t[:], x_t[:])
                nc.vector.reduce_sum(ss8[:, bi : bi + 1], sq[:], axis=X)
            xn8 = persist.tile([128, nb], f32)
            nc.scalar.activation(xn8[:], ss8[:], Act.Sqrt)
            nc.vector.reciprocal(xinv8[:], xn8[:])
            xinvS8 = persist.tile([128, nb], f32)
            nc.vector.tensor_scalar_mul(xinvS8[:], xinv8[:], S)
            for bi in range(nb):
                # xhatS (bf16) and its transpose into xT
                xhS = xsc.tile([128, d], bf16)
                nc.scalar.mul(xhS[:], x_tiles[bi][:], xinvS8[:, bi : bi + 1])
                for k in range(nkt):
                    ptile = po.tile([128, 128], bf16, tag="ops", name="ptile")
                    nc.tensor.transpose(
                        ptile[:], xhS[:, k * 128 : (k + 1) * 128], ident_bf[:]
                    )
                    col = k * b + bi * 128
                    nc.vector.tensor_copy(xT[:, col : col + 128], ptile[:])

            # ---- Phase B: main loop over c-groups ----
            for cg in range(nct // ncg):
                # per-group weight prep (squares on DVE in 2x bf16 mode;
                # norm fold on gpsimd so DVE keeps cycles for PSUM copies)
                # weights arrive as c-tile PAIRS: 2 KB strided rows keep
                # all 16 DMA engines balanced but halve descriptor count
                npair = (ncg + 1) // 2
                wt_fp = []  # [pj][k] pair tiles
                wt2_p = []
                for pj in range(npair):
                    w = min(2 * tc, (ncg - 2 * pj) * tc)
                    c0 = (cg * ncg + 2 * pj) * tc
                    fpk = []
                    p2k = []
                    for k in range(nkt):
                        wt_f = wstream.tile(
                            [128, 2 * tc], bf16, tag="wt_f", name="wt_f",
                            bufs=4 * ncg,
                        )
                        nc.sync.dma_start(
                            wt_f[:, :w], wt_ext[k, :, c0 : c0 + w]
                        )
                        wt2 = wstream.tile(
                            [128, 2 * tc], bf16, tag="wt2", name="wt2", bufs=8
                        )
                        nc.vector.tensor_tensor(
                            wt2[:, :w], wt_f[:, :w], wt_f[:, :w], Op.mult
                        )
                        fpk.append(wt_f)
                        p2k.append(wt2)
                    wt_fp.append(fpk)
                    wt2_p.append(p2k)
                def _sl(ci5):
                    return slice((ci5 % 2) * tc, (ci5 % 2 + 1) * tc)
                wt_f_g = [
                    [wt_fp[ci5 // 2][k][:, _sl(ci5)] for k in range(nkt)]
                    for ci5 in range(ncg)
                ]
                wt2_g = [
                    [wt2_p[ci5 // 2][k][:, _sl(ci5)] for k in range(nkt)]
                    for ci5 in range(ncg)
                ]
                # winv = 1/sqrt(n2): ACT sqrt + fast DVE reciprocal
                wlog_g = []
                for ci5 in range(ncg):
                    nps = pn.tile([128, tc], f32, tag="nps", name="nps")
                    for k in range(nkt):
                        nc.tensor.matmul(
                            nps[:],
                            lhsT=ones_bf[:],
                            rhs=wt2_g[ci5][k],
                            start=(k == 0),
                            stop=(k == nkt - 1),
                        )
                    wlog = winvp.tile([128, tc], f32, tag="wlog", name="wlog")
                    nc.scalar.activation(wlog[:], nps[:], Act.Sqrt)
                    wlog_g.append(wlog)
                winv_g = []
                for ci5 in range(ncg):
                    winv = winvp.tile([128, tc], f32, tag="winv", name="winv")
                    nc.vector.reciprocal_approx_fast(winv[:], wlog_g[ci5][:])
                    winv_g.append(winv)
                # fold the column norm into the bf16 weights (gpsimd)
                wt_bf_g = []
                for ci5 in range(ncg):
                    wt_bf_k = []
                    for k in range(nkt):
                        wt_bf = wbf.tile(
                            [128, tc], bf16, tag="wt_bf", name="wt_bf"
                        )
                        presc = nc.vector if cg == 0 else nc.gpsimd
                        presc.tensor_tensor(
                            wt_bf[:], wt_f_g[ci5][k], winv_g[ci5][:], Op.mult
                        )
                        wt_bf_k.append(wt_bf)
                    wt_bf_g.append(wt_bf_k)
                # matmuls: k-outer keeps the stationary operand loaded
                for bi in range(nb):
                    ops_g = [
                        po.tile([128, tc], f32, tag="ops", name="ops")
                        for _ in range(ncg)
                    ]
                    for k in range(nkt):
                        col = k * b + bi * 128
                        for ci5 in range(ncg):
                            nc.tensor.matmul(
                                ops_g[ci5][:],
                                lhsT=xT[:, col : col + 128],
                                rhs=wt_bf_g[ci5][k][:],
                                start=(k == 0),
                                stop=(k == nkt - 1),
                            )
                    stw = stage.tile([128, ncg * tc], f32, name="stw")
                    for ci5 in range(ncg):
                        dst = stw[:, ci5 * tc : (ci5 + 1) * tc]
                        if ci5 < 2:
                            nc.vector.tensor_copy(dst, ops_g[ci5][:])
                        else:
                            nc.scalar.copy(dst, ops_g[ci5][:])
                    nc.sync.dma_start(out_blocks[bi][cg], stw[:])

                if cg == min(1, nct // ncg - 1):
                    # ---- Phase A2: wsel / margin path (feeds only the scatters) ----
                    for bi in range(nb):
                        rsl = slice(bi * 128, (bi + 1) * 128)
                        ws_t = xin.tile([128, d], f32, tag="ws_t", name="ws_t")
                        nc.sync.dma_start(ws_t[:], wsel_ext[rsl, :])
                        sq2 = xsc.tile([128, d], f32)
                        nc.vector.tensor_mul(sq2[:], ws_t[:], ws_t[:])
                        nc.vector.reduce_sum(wss8[:, bi : bi + 1], sq2[:], axis=X)
                        pr = xsc.tile([128, d], f32)
                        nc.vector.tensor_mul(pr[:], x_tiles[bi][:], ws_t[:])
                        nc.vector.reduce_sum(rawdot8[:, bi : bi + 1], pr[:], axis=X)
                    wn8 = persist.tile([128, nb], f32)
                    nc.scalar.activation(wn8[:], wss8[:], Act.Sqrt)
                    nc.vector.reciprocal(wsinv8[:], wn8[:])

                    # margin math on [128, nb]
                    tgt8 = persist.tile([128, nb], f32)
                    nc.vector.tensor_mul(tgt8[:], rawdot8[:], xinv8[:])
                    nc.vector.tensor_mul(tgt8[:], tgt8[:], wsinv8[:])
                    tsq = persist.tile([128, nb], f32)
                    nc.vector.tensor_mul(tsq[:], tgt8[:], tgt8[:])
                    om = persist.tile([128, nb], f32)
                    nc.vector.tensor_scalar(om[:], tsq[:], -1.0, 1.0, Op.mult, Op.add)
                    nc.vector.tensor_scalar_max(om[:], om[:], 0.0)
                    sine8 = persist.tile([128, nb], f32)
                    nc.scalar.activation(sine8[:], om[:], Act.Sqrt)
                    phi8 = persist.tile([128, nb], f32)
                    nc.vector.tensor_scalar_mul(phi8[:], tgt8[:], COS_M)
                    ssin8 = persist.tile([128, nb], f32)
                    nc.vector.tensor_scalar_mul(ssin8[:], sine8[:], SIN_M)
                    nc.vector.tensor_sub(phi8[:], phi8[:], ssin8[:])
                    mask8 = persist.tile([128, nb], mybir.dt.uint8)
                    nc.vector.tensor_scalar(mask8[:], tgt8[:], 0.0, None, Op.is_gt)
                    selv8 = persist.tile([128, nb], f32)
                    nc.vector.select(selv8[:], mask8[:], phi8[:], tgt8[:])
                    nc.vector.tensor_scalar_mul(newv8[:], selv8[:], S)
                    # flat offsets: p*(ncg*tc) + rel2 (group term + OOB
                    # sentinel already host-encoded)
                    o1 = persist.tile([128, nb], f32)
                    nc.vector.tensor_add(o1[:], iota_f[:], rel_f[:])
                    nc.vector.tensor_copy(offs_i[:], o1[:])




            # ---- Phase C: scatter the margin values ----
            # per-block scatters depend only on their block's bulk DMAs
            # (ordering via Tile's dependency tracking on the out tensor APs)
            for bi in range(nb):
                out_flat = out_blocks[bi][:].rearrange(
                    "g r (c one) -> (g r c) one", one=1
                )
                nc.gpsimd.indirect_dma_start(
                    out=out_flat,
                    out_offset=bass.IndirectOffsetOnAxis(
                        ap=offs_i[:, bi : bi + 1], axis=0
                    ),
                    in_=newv8[:, bi : bi + 1],
                    in_offset=None,
                    bounds_check=128 * cs - 1,
                    oob_is_err=False,
                )

    nc.compile()
    return nc


def host_prep(cfg: Cfg, input, label, weight):
    x = np.ascontiguousarray(np.asarray(input, dtype=np.float32))
    w = np.asarray(weight, dtype=np.float32)
    lab = np.asarray(label).astype(np.int64)
    wsel = np.ascontiguousarray(w[lab])
    wt_all = np.ascontiguousarray(w.T)  # [D, C], relayout only
    in_maps = []
    for core in range(cfg.ncores):
        sl = slice(core * cfg.cs, (core + 1) * cfg.cs)
        wt = (
            np.ascontiguousarray(wt_all[:, sl])
            .reshape(cfg.nkt, 128, cfg.cs)
            .astype(ml_dtypes.bfloat16)
        )
        rel = lab - core * cfg.cs
        gw = cfg.ncg * cfg.tc
        inb = (rel >= 0) & (rel < cfg.cs)
        relc = np.where(inb, rel, 0)
        rel2 = np.where(
            inb, (relc // gw) * (128 * gw) + relc % gw, 2**30
        ).astype(np.int32)
        labrel = np.ascontiguousarray(rel2.reshape(cfg.nb, 128).T)
        in_maps.append({"x": x, "wt": wt, "wsel": wsel, "labrel": labrel})
    return in_maps


def run(cfg: Cfg, nc, in_maps, **kw):
    from concourse.bass_utils import run_bass_kernel_spmd

    try:
        res = run_bass_kernel_spmd(
            nc, in_maps, core_ids=list(range(cfg.ncores)), **kw
        )
    except Exception:
        # rare transient device faults have been observed; retry once
        res = run_bass_kernel_spmd(
            nc, in_maps, core_ids=list(range(cfg.ncores)), **kw
        )
    out = np.empty((cfg.b, cfg.c), dtype=np.float32)
    for c in range(cfg.ncores):
        for bi in range(cfg.nb):
            blk = res.results[c][f"out{bi}"]  # [ngroups, 128, gw]
            out[bi * 128 : (bi + 1) * 128, c * cfg.cs : (c + 1) * cfg.cs] = (
                blk.transpose(1, 0, 2).reshape(128, cfg.cs)
            )
    return out, res


_cache = {}


def kernel(input, label, weight):
    cfg = Cfg()
    if cfg not in _cache:
        _cache[cfg] = build(cfg)
    in_maps = host_prep(cfg, input, label, weight)
    out, _ = run(cfg, _cache[cfg], in_maps)
    return out



# revision 11
# speedup vs baseline: 1.2761x; 1.2761x over previous
"""AAM-Softmax (ArcFace) logits kernel for Trainium2, 8 NeuronCores.

Math (per reference):
    cosine = l2norm(input) @ l2norm(weight).T            # [B, C]
    tgt    = cosine[i, label[i]]
    phi    = tgt*cos(m) - sqrt(1-tgt^2)*sin(m)
    out    = S * cosine, except out[i, label[i]] = S * where(tgt>0, phi, tgt)

Sharding: weight/cosine column-sharded over 8 cores (vocab parallel);
input + labels replicated.  Core k owns classes [k*CS, (k+1)*CS).

v3 pipeline (dense-queue design; output stored bf16, host upcasts —
rel tolerance 2e-2 dwarfs bf16 rounding, and it halves the dominant
HBM write traffic):
  - weights ship as host-relayouted W.T shard [2, 128, CS] bf16; one
    [128, 2500]-wide DMA per (group, k), staggered 3 groups ahead.
  - per group: squares (DVE, one op per k), column-norm^2 via
    ones-matmul broadcast (PE, PSUM), sqrt (ACT) + fast reciprocal
    (DVE) -> winv [128, 500] broadcast tiles.
  - winv is folded POST-matmul during the PSUM->staging copy for 3 of
    5 c-tiles (tensor_tensor on DVE/GpSimd); the other 2 c-tiles are
    pre-folded into bf16 weights (DVE) so the scalar/ACT engine can
    drain them with plain copies. This spreads the PSUM drain over 3
    engines and eliminates the old full-weight fold pass.
  - group prep for g+2 is emitted inside group g's bi-loop so every
    engine queue stays dense; x/margin prep overlaps the weight
    stream; margin scatter is a single merged indirect DMA at the
    tail (flat offsets host-encoded per label, OOB sentinel for
    out-of-shard rows).
"""

import sys

if "/opt/trn_rl_repo" not in sys.path:
    sys.path.insert(0, "/opt/trn_rl_repo")

from dataclasses import dataclass

import ml_dtypes
import numpy as np

S = 50.0
MARGIN = 0.5
COS_M = float(np.cos(MARGIN))
SIN_M = float(np.sin(MARGIN))
OOB = 2**30  # exact in f32, > any valid flat offset


@dataclass(frozen=True)
class Cfg:
    b: int = 1024
    d: int = 256
    c: int = 100000
    ncores: int = 8
    tc: int = 500

    @property
    def cs(self):
        return self.c // self.ncores

    @property
    def nb(self):
        return self.b // 128

    @property
    def nkt(self):
        return self.d // 128

    @property
    def nct(self):
        return self.cs // self.tc

    @property
    def ncg(self):
        return 5  # c-tiles per group

    @property
    def ngr(self):
        return self.nct // self.ncg  # groups

    @property
    def gw(self):
        return self.ncg * self.tc  # group width in classes


def build(cfg: Cfg):
    import concourse.bass as bass
    import concourse.tile as tile
    from concourse import bacc, mybir
    from concourse.masks import make_identity

    f32 = mybir.dt.float32
    bf16 = mybir.dt.bfloat16
    i32 = mybir.dt.int32
    X = mybir.AxisListType.X
    Op = mybir.AluOpType
    Act = mybir.ActivationFunctionType

    b, d, cs, tc = cfg.b, cfg.d, cfg.cs, cfg.tc
    nb, nkt, ncg, ngr, gw = cfg.nb, cfg.nkt, cfg.ncg, cfg.ngr, cfg.gw

    nc = bacc.Bacc(
        "TRN2", target_bir_lowering=False, debug=False, num_devices=cfg.ncores
    )

    x_ext = nc.dram_tensor("x", [b, d], f32, kind="ExternalInput")
    wt_ext = nc.dram_tensor("wt", [nkt, 128, cs], bf16, kind="ExternalInput")
    wsel_ext = nc.dram_tensor("wsel", [b, d], f32, kind="ExternalInput")
    labrel_ext = nc.dram_tensor("labrel", [128, nb], i32, kind="ExternalInput")
    # one output tensor: [bi, group, partition, group-cols]; host
    # de-interleaves groups on assembly
    out_ext = nc.dram_tensor(
        "out", [nb, ngr, 128, gw], bf16, kind="ExternalOutput"
    )

    with tile.TileContext(nc) as tc_:
        with (
            tc_.tile_pool(name="const", bufs=1) as constp,
            tc_.tile_pool(name="persist", bufs=1) as persist,
            tc_.tile_pool(name="xin", bufs=nb) as xin,
            tc_.tile_pool(name="wsin", bufs=nb) as wsin,
            tc_.tile_pool(name="xsc", bufs=2) as xsc,
            tc_.tile_pool(name="wstream", bufs=8) as wstream,
            tc_.tile_pool(name="wt2p", bufs=4) as wt2p,
            tc_.tile_pool(name="wlogp", bufs=2) as wlogp,
            tc_.tile_pool(name="winvp", bufs=15) as winvp,
            tc_.tile_pool(name="wbfp", bufs=12) as wbfp,
            tc_.tile_pool(name="stage", bufs=5) as stage,
            tc_.tile_pool(name="pn", bufs=2, space="PSUM") as pn,
            tc_.tile_pool(name="po", bufs=6, space="PSUM") as po,
        ):
            ident_bf = constp.tile([128, 128], bf16)
            make_identity(nc, ident_bf[:])
            ones_bf = constp.tile([128, 128], bf16)
            nc.vector.memset(ones_bf[:], 1.0)

            # persistent tensors
            xT = persist.tile([128, nkt * b], bf16)  # [d-half][k*b + i]
            labrel_t = persist.tile([128, nb], i32)
            rel_f = persist.tile([128, nb], f32)
            iota_i = persist.tile([128, nb], i32)
            iota_f = persist.tile([128, nb], f32)
            ss8 = persist.tile([128, nb], f32)
            wss8 = persist.tile([128, nb], f32)
            rawdot8 = persist.tile([128, nb], f32)
            xnS8 = persist.tile([128, nb], f32)
            xinvS8 = persist.tile([128, nb], f32)
            xinv8 = persist.tile([128, nb], f32)
            wn8 = persist.tile([128, nb], f32)
            wsinv8 = persist.tile([128, nb], f32)
            newv8 = persist.tile([128, nb], bf16)
            o1 = persist.tile([128, nb], f32)
            offs_i = persist.tile([128, nb], i32)

            # ---- weight stream state ----
            wt_f_g = {}  # g -> [k] tiles [128, gw] bf16
            wt2_g = {}  # g -> [k] tiles [128, gw] bf16
            winv_g = {}  # g -> [ci5] tiles [128, tc] f32
            wbf_g = {}  # g -> {ci5: [k] tiles [128, tc] bf16}

            def wt_dma(g):
                ks = []
                for k in range(nkt):
                    wt_f = wstream.tile(
                        [128, gw], bf16, tag="wt_f", name="wt_f", bufs=8
                    )
                    nc.sync.dma_start(
                        wt_f[:], wt_ext[k, :, g * gw : (g + 1) * gw]
                    )
                    ks.append(wt_f)
                wt_f_g[g] = ks

            def prep(g):
                # squares (one wide DVE op per k; bf16 in+out -> 2x mode)
                w2s = []
                for k in range(nkt):
                    wt2 = wt2p.tile(
                        [128, gw], bf16, tag="wt2", name="wt2", bufs=4
                    )
                    nc.vector.tensor_tensor(
                        wt2[:], wt_f_g[g][k][:], wt_f_g[g][k][:], Op.mult
                    )
                    w2s.append(wt2)
                wt2_g[g] = w2s
                # column norm^2 broadcast over partitions, then
                # winv = 1/sqrt(n2) via ACT sqrt + DVE fast reciprocal
                winvs = []
                for ci5 in range(ncg):
                    sl = slice(ci5 * tc, (ci5 + 1) * tc)
                    nps = pn.tile([128, tc], f32, tag="nps", name="nps")
                    for k in range(nkt):
                        nc.tensor.matmul(
                            nps[:],
                            lhsT=ones_bf[:],
                            rhs=w2s[k][:, sl],
                            start=(k == 0),
                            stop=(k == nkt - 1),
                        )
                    wlog = wlogp.tile(
                        [128, tc], f32, tag="wlog", name="wlog", bufs=2
                    )
                    nc.scalar.activation(wlog[:], nps[:], Act.Sqrt)
                    winv = winvp.tile(
                        [128, tc], f32, tag="winv", name="winv", bufs=15
                    )
                    nc.vector.reciprocal_approx_fast(winv[:], wlog[:])
                    winvs.append(winv)
                winv_g[g] = winvs
                # pre-fold winv into bf16 weights for the ACT-drained
                # c-tiles (ci5 0,1,2) on gpsimd (SBUF-only engine); the
                # rest (ci5 3,4) fold during the DVE copy
                pf = {}
                for ci5 in range(3):
                    sl = slice(ci5 * tc, (ci5 + 1) * tc)
                    ks = []
                    for k in range(nkt):
                        wt_bf = wbfp.tile(
                            [128, tc], bf16, tag="wt_bf", name="wt_bf",
                            bufs=18,
                        )
                        nc.gpsimd.tensor_tensor(
                            wt_bf[:], wt_f_g[g][k][:, sl], winvs[ci5][:],
                            Op.mult,
                        )
                        ks.append(wt_bf)
                    pf[ci5] = ks
                wbf_g[g] = pf

            # ---- prologue ----
            wt_dma(0)
            nc.sync.dma_start(labrel_t[:], labrel_ext[:])
            x_tiles = []
            for bi in range(nb):
                x_t = xin.tile([128, d], f32, tag="x_t", name="x_t", bufs=nb)
                nc.sync.dma_start(x_t[:], x_ext[bi * 128 : (bi + 1) * 128, :])
                x_tiles.append(x_t)
            wt_dma(1)
            wt_dma(2)
            ws_tiles = []
            for bi in range(nb):
                ws_t = wsin.tile(
                    [128, d], f32, tag="ws_t", name="ws_t", bufs=nb
                )
                nc.sync.dma_start(
                    ws_t[:], wsel_ext[bi * 128 : (bi + 1) * 128, :]
                )
                ws_tiles.append(ws_t)

            # flat scatter offsets: p*gw device term + host-encoded
            # (bi*ngr+g)*128*gw + col term (OOB sentinel for off-shard)
            nc.gpsimd.iota(
                iota_i[:], pattern=[[0, nb]], base=0, channel_multiplier=gw
            )
            nc.vector.tensor_copy(iota_f[:], iota_i[:])
            nc.vector.tensor_copy(rel_f[:], labrel_t[:])
            nc.vector.tensor_add(o1[:], iota_f[:], rel_f[:])
            nc.vector.tensor_copy(offs_i[:], o1[:])

            # x prep: row sumsq via ACT square+accumulate
            for bi in range(nb):
                sq = xsc.tile([128, d], f32, tag="sq", name="sq", bufs=2)
                nc.scalar.activation(
                    sq[:], x_tiles[bi][:], Act.Square,
                    accum_out=ss8[:, bi : bi + 1],
                )
            # xnS = ||x||/S; xinvS = S/||x||
            nc.scalar.activation(
                xnS8[:], ss8[:], Act.Sqrt, scale=1.0 / (S * S)
            )
            nc.vector.reciprocal(xinvS8[:], xnS8[:])
            nc.vector.tensor_scalar_mul(xinv8[:], xinvS8[:], 1.0 / S)
            for bi in range(nb):
                xhs = xsc.tile([128, d], bf16, tag="xhs", name="xhs", bufs=2)
                nc.scalar.mul(xhs[:], x_tiles[bi][:], xinvS8[:, bi : bi + 1])
                for k in range(nkt):
                    ptile = po.tile([128, 128], bf16, tag="ops", name="ptile")
                    nc.tensor.transpose(
                        ptile[:], xhs[:, k * 128 : (k + 1) * 128], ident_bf[:]
                    )
                    col = k * b + bi * 128
                    nc.vector.tensor_copy(xT[:, col : col + 128], ptile[:])
                    # (DVE: ptile is PSUM; gpsimd cannot read PSUM)

            prep(0)
            prep(1)

            # ---- main loop ----
            for g in range(ngr):
                for bi in range(nb):
                    if bi == 1:
                        if g == 0:
                            wt_dma(3)
                        elif g == 1:
                            wt_dma(4)
                        if g + 2 < ngr:
                            prep(g + 2)
                        if g == 3:
                            # wsel row sumsq (ACT, accumulate)
                            for bj in range(nb):
                                sq = xsc.tile(
                                    [128, d], f32, tag="sq", name="sq", bufs=2
                                )
                                nc.scalar.activation(
                                    sq[:], ws_tiles[bj][:], Act.Square,
                                    accum_out=wss8[:, bj : bj + 1],
                                )
                        if g == 4:
                            # raw dot x . wsel per row (mults on gpsimd,
                            # reduce on DVE)
                            for bj in range(nb):
                                pr = xsc.tile(
                                    [128, d], f32, tag="pr", name="pr", bufs=2
                                )
                                nc.gpsimd.tensor_tensor(
                                    pr[:], x_tiles[bj][:], ws_tiles[bj][:],
                                    Op.mult,
                                )
                                nc.vector.reduce_sum(
                                    rawdot8[:, bj : bj + 1], pr[:], axis=X
                                )
                            nc.scalar.activation(wn8[:], wss8[:], Act.Sqrt)
                            nc.vector.reciprocal(wsinv8[:], wn8[:])

                    ops_g = [
                        po.tile([128, tc], f32, tag="ops", name="ops")
                        for _ in range(ncg)
                    ]
                    for k in range(nkt):
                        col = k * b + bi * 128
                        lhs = xT[:, col : col + 128]
                        for ci5 in range(ncg):
                            if ci5 < 3:
                                rhs = wbf_g[g][ci5][k][:]
                            else:
                                rhs = wt_f_g[g][k][
                                    :, ci5 * tc : (ci5 + 1) * tc
                                ]
                            nc.tensor.matmul(
                                ops_g[ci5][:],
                                lhsT=lhs,
                                rhs=rhs,
                                start=(k == 0),
                                stop=(k == nkt - 1),
                            )
                    stw = stage.tile(
                        [128, gw], bf16, tag="stw", name="stw", bufs=5
                    )
                    for ci5 in range(ncg):
                        dst = stw[:, ci5 * tc : (ci5 + 1) * tc]
                        if ci5 < 2 or (ci5 == 2 and bi % 4 != 3):
                            nc.scalar.copy(dst, ops_g[ci5][:])
                        elif ci5 == 2:
                            nc.vector.tensor_copy(dst, ops_g[ci5][:])
                        else:
                            nc.vector.tensor_tensor(
                                dst, ops_g[ci5][:], winv_g[g][ci5][:], Op.mult
                            )
                    nc.sync.dma_start(out_ext[bi, g], stw[:])

            # ---- margin tail (feeds only the final scatter) ----
            tgt8 = persist.tile([128, nb], f32)
            nc.vector.tensor_mul(tgt8[:], rawdot8[:], xinv8[:])
            nc.vector.tensor_mul(tgt8[:], tgt8[:], wsinv8[:])
            tsq = persist.tile([128, nb], f32)
            nc.vector.tensor_mul(tsq[:], tgt8[:], tgt8[:])
            om = persist.tile([128, nb], f32)
            nc.vector.tensor_scalar(om[:], tsq[:], -1.0, 1.0, Op.mult, Op.add)
            nc.vector.tensor_scalar_max(om[:], om[:], 0.0)
            sine8 = persist.tile([128, nb], f32)
            nc.scalar.activation(sine8[:], om[:], Act.Sqrt)
            phi8 = persist.tile([128, nb], f32)
            nc.vector.tensor_scalar_mul(phi8[:], tgt8[:], COS_M)
            ssin8 = persist.tile([128, nb], f32)
            nc.vector.tensor_scalar_mul(ssin8[:], sine8[:], SIN_M)
            nc.vector.tensor_sub(phi8[:], phi8[:], ssin8[:])
            mask8 = persist.tile([128, nb], mybir.dt.uint8)
            nc.vector.tensor_scalar(mask8[:], tgt8[:], 0.0, None, Op.is_gt)
            selv8 = persist.tile([128, nb], f32)
            nc.vector.select(selv8[:], mask8[:], phi8[:], tgt8[:])
            nc.vector.tensor_scalar_mul(newv8[:], selv8[:], S)

            out_flat = out_ext[:].rearrange(
                "a g r (c one) -> (a g r c) one", one=1
            )
            for bi in range(nb):
                nc.gpsimd.indirect_dma_start(
                    out=out_flat,
                    out_offset=bass.IndirectOffsetOnAxis(
                        ap=offs_i[:, bi : bi + 1], axis=0
                    ),
                    in_=newv8[:, bi : bi + 1],
                    in_offset=None,
                    bounds_check=nb * ngr * 128 * gw - 1,
                    oob_is_err=False,
                )

    nc.compile()
    return nc


def host_prep(cfg: Cfg, input, label, weight):
    x = np.ascontiguousarray(np.asarray(input, dtype=np.float32))
    w = np.asarray(weight, dtype=np.float32)
    lab = np.asarray(label).astype(np.int64)
    wsel = np.ascontiguousarray(w[lab])
    wt_all = np.ascontiguousarray(w.T)  # [D, C], relayout only
    in_maps = []
    for core in range(cfg.ncores):
        sl = slice(core * cfg.cs, (core + 1) * cfg.cs)
        wt = (
            np.ascontiguousarray(wt_all[:, sl])
            .reshape(cfg.nkt, 128, cfg.cs)
            .astype(ml_dtypes.bfloat16)
        )
        rel = lab - core * cfg.cs
        inb = (rel >= 0) & (rel < cfg.cs)
        relc = np.where(inb, rel, 0)
        bi_of = np.arange(cfg.b) // 128
        g_of = relc // cfg.gw
        col_of = relc % cfg.gw
        rel2 = np.where(
            inb, (bi_of * cfg.ngr + g_of) * (128 * cfg.gw) + col_of, OOB
        ).astype(np.int32)
        labrel = np.ascontiguousarray(rel2.reshape(cfg.nb, 128).T)
        in_maps.append({"x": x, "wt": wt, "wsel": wsel, "labrel": labrel})
    return in_maps


def run(cfg: Cfg, nc, in_maps, **kw):
    from concourse.bass_utils import run_bass_kernel_spmd

    try:
        res = run_bass_kernel_spmd(
            nc, in_maps, core_ids=list(range(cfg.ncores)), **kw
        )
    except Exception:
        # rare transient device faults have been observed; retry once
        res = run_bass_kernel_spmd(
            nc, in_maps, core_ids=list(range(cfg.ncores)), **kw
        )
    out = np.empty((cfg.b, cfg.c), dtype=np.float32)
    for c in range(cfg.ncores):
        blk = res.results[c]["out"]  # [nb, ngr, 128, gw] bf16
        full = (
            blk.transpose(0, 2, 1, 3)
            .reshape(cfg.b, cfg.cs)
            .astype(np.float32)
        )
        out[:, c * cfg.cs : (c + 1) * cfg.cs] = full
    return out, res


_cache = {}


def kernel(input, label, weight):
    cfg = Cfg()
    if cfg not in _cache:
        _cache[cfg] = build(cfg)
    in_maps = host_prep(cfg, input, label, weight)
    out, _ = run(cfg, _cache[cfg], in_maps)
    return out


# revision 16
# speedup vs baseline: 1.3988x; 1.0961x over previous
"""AAM-Softmax (ArcFace) logits kernel for Trainium2, 8 NeuronCores.

Math (per reference):
    cosine = l2norm(input) @ l2norm(weight).T            # [B, C]
    tgt    = cosine[i, label[i]]
    phi    = tgt*cos(m) - sqrt(1-tgt^2)*sin(m)
    out    = S * cosine, except out[i, label[i]] = S * where(tgt>0, phi, tgt)

Sharding: weight/cosine column-sharded over 8 cores (vocab parallel);
input + labels replicated.  Core k owns classes [k*CS, (k+1)*CS).

v4 pipeline (dense-queue design; output stored bf16, host upcasts —
rel tolerance 2e-2 dwarfs bf16 rounding, and it halves the dominant
HBM write traffic):
  - x ships twice: rows f32 (for norms + margin dot) and pre-transposed
    x.T bf16 (pure host relayout) straight into the matmul operand —
    no on-device transposes, so the PE starts immediately.
  - per-sample scale S/||x|| is folded into the PSUM->staging copies as
    a per-partition scalar (ACT scalar.mul / DVE tensor_scalar_mul);
    per-class 1/||w|| is pre-folded into the bf16 weights on gpsimd
    (DVE for group 0 to shorten the prologue chain).
  - weights ship as host-relayouted W.T shard [2, 128, CS] bf16; one
    [128, 2500]-wide DMA per (group, k), staggered 3 groups ahead;
    per group: squares (DVE) -> column-norm^2 ones-matmul broadcast
    (PE) -> sqrt (ACT) -> fast reciprocal (DVE) -> prefold.
  - group prep for g+2 is emitted inside group g's bi-loop so every
    engine queue stays dense; margin math runs early in the last
    group, and the per-block margin scatters (flat offsets
    host-encoded, OOB sentinel for off-shard rows) are emitted right
    after each block's final bulk DMA so they pipeline with the drain.
"""

import sys

if "/opt/trn_rl_repo" not in sys.path:
    sys.path.insert(0, "/opt/trn_rl_repo")

from dataclasses import dataclass

import ml_dtypes
import numpy as np

S = 50.0
MARGIN = 0.5
COS_M = float(np.cos(MARGIN))
SIN_M = float(np.sin(MARGIN))
OOB = 2**30  # exact in f32, > any valid flat offset


@dataclass(frozen=True)
class Cfg:
    b: int = 1024
    d: int = 256
    c: int = 100000
    ncores: int = 8
    tc: int = 500

    @property
    def cs(self):
        return self.c // self.ncores

    @property
    def nb(self):
        return self.b // 128

    @property
    def nkt(self):
        return self.d // 128

    @property
    def nct(self):
        return self.cs // self.tc

    @property
    def ncg(self):
        return 5  # c-tiles per group

    @property
    def ngr(self):
        return self.nct // self.ncg  # groups

    @property
    def gw(self):
        return self.ncg * self.tc  # group width in classes


def build(cfg: Cfg):
    import concourse.bass as bass
    import concourse.tile as tile
    from concourse import bacc, mybir

    f32 = mybir.dt.float32
    bf16 = mybir.dt.bfloat16
    i32 = mybir.dt.int32
    X = mybir.AxisListType.X
    Op = mybir.AluOpType
    Act = mybir.ActivationFunctionType

    b, d, cs, tc = cfg.b, cfg.d, cfg.cs, cfg.tc
    nb, nkt, ncg, ngr, gw = cfg.nb, cfg.nkt, cfg.ncg, cfg.ngr, cfg.gw

    nc = bacc.Bacc(
        "TRN2", target_bir_lowering=False, debug=False, num_devices=cfg.ncores
    )

    x_ext = nc.dram_tensor("x", [b, d], f32, kind="ExternalInput")
    xt_ext = nc.dram_tensor("xt", [nkt, 128, b], bf16, kind="ExternalInput")
    wt_ext = nc.dram_tensor("wt", [nkt, 128, cs], bf16, kind="ExternalInput")
    wsel_ext = nc.dram_tensor("wsel", [b, d], f32, kind="ExternalInput")
    labrel_ext = nc.dram_tensor("labrel", [128, nb], i32, kind="ExternalInput")
    # per-block output tensors (indirect-DMA dynamic APs need offset 0):
    # [group, partition, group-cols]; host de-interleaves on assembly
    out_blocks = [
        nc.dram_tensor(f"out{bi}", [ngr, 128, gw], bf16,
                       kind="ExternalOutput")
        for bi in range(nb)
    ]

    with tile.TileContext(nc) as tc_:
        with (
            tc_.tile_pool(name="const", bufs=1) as constp,
            tc_.tile_pool(name="persist", bufs=1) as persist,
            tc_.tile_pool(name="xin", bufs=nb) as xin,
            tc_.tile_pool(name="wsin", bufs=nb) as wsin,
            tc_.tile_pool(name="xsc", bufs=2) as xsc,
            tc_.tile_pool(name="wstream", bufs=8) as wstream,
            tc_.tile_pool(name="wt2p", bufs=4) as wt2p,
            tc_.tile_pool(name="wlogp", bufs=2) as wlogp,
            tc_.tile_pool(name="winvp", bufs=6) as winvp,
            tc_.tile_pool(name="wbfp", bufs=18) as wbfp,
            tc_.tile_pool(name="stage", bufs=5) as stage,
            tc_.tile_pool(name="pn", bufs=2, space="PSUM") as pn,
            tc_.tile_pool(name="po", bufs=6, space="PSUM") as po,
        ):
            ones_bf = constp.tile([128, 128], bf16)
            nc.vector.memset(ones_bf[:], 1.0)

            # persistent tensors
            xT = persist.tile([128, nkt * b], bf16)  # [d-half][k*b + i]
            labrel_t = persist.tile([128, nb], i32)
            rel_f = persist.tile([128, nb], f32)
            iota_i = persist.tile([128, nb], i32)
            iota_f = persist.tile([128, nb], f32)
            ss8 = persist.tile([128, nb], f32)
            wss8 = persist.tile([128, nb], f32)
            rawdot8 = persist.tile([128, nb], f32)
            xnS8 = persist.tile([128, nb], f32)
            xinvS8 = persist.tile([128, nb], f32)
            xinv8 = persist.tile([128, nb], f32)
            wn8 = persist.tile([128, nb], f32)
            wsinv8 = persist.tile([128, nb], f32)
            newv8 = persist.tile([128, nb], bf16)
            o1 = persist.tile([128, nb], f32)
            offs_i = persist.tile([128, nb], i32)

            # ---- weight stream state ----
            wt_f_g = {}  # g -> [k] tiles [128, gw] bf16
            winv_g = {}  # g -> (wpair_a [128,1000], wnar [128,500], wpair_b)
            wbf_g = {}  # g -> [k] -> (wide_a, narrow, wide_b) folded bf16

            def wt_dma(g):
                ks = []
                for k in range(nkt):
                    wt_f = wstream.tile(
                        [128, gw], bf16, tag="wt_f", name="wt_f", bufs=8
                    )
                    nc.sync.dma_start(
                        wt_f[:], wt_ext[k, :, g * gw : (g + 1) * gw]
                    )
                    ks.append(wt_f)
                wt_f_g[g] = ks

            def prep(g, fold_engine):
                # squares (one wide DVE op per k; bf16 in+out -> 2x mode)
                w2s = []
                for k in range(nkt):
                    wt2 = wt2p.tile(
                        [128, gw], bf16, tag="wt2", name="wt2", bufs=4
                    )
                    nc.vector.tensor_tensor(
                        wt2[:], wt_f_g[g][k][:], wt_f_g[g][k][:], Op.mult
                    )
                    w2s.append(wt2)
                # column norm^2 broadcast over partitions, then
                # winv = 1/sqrt(n2) via ACT sqrt + DVE fast reciprocal.
                # winv lives in pair tiles so reciprocal + prefold run
                # as wide ops: wa = ci5 0,1; wn = ci5 2; wb = ci5 3,4.
                wla = wlogp.tile([128, 2 * tc], f32, tag="wla", name="wla",
                                 bufs=2)
                wln = wlogp.tile([128, tc], f32, tag="wln", name="wln",
                                 bufs=2)
                wlb = wlogp.tile([128, 2 * tc], f32, tag="wlb", name="wlb",
                                 bufs=2)
                dsts = [
                    wla[:, 0:tc], wla[:, tc : 2 * tc], wln[:],
                    wlb[:, 0:tc], wlb[:, tc : 2 * tc],
                ]
                for ci5 in range(ncg):
                    sl = slice(ci5 * tc, (ci5 + 1) * tc)
                    nps = pn.tile([128, tc], f32, tag="nps", name="nps")
                    for k in range(nkt):
                        nc.tensor.matmul(
                            nps[:],
                            lhsT=ones_bf[:],
                            rhs=w2s[k][:, sl],
                            start=(k == 0),
                            stop=(k == nkt - 1),
                        )
                    nc.scalar.activation(dsts[ci5], nps[:], Act.Sqrt)
                wa = winvp.tile([128, 2 * tc], f32, tag="wa", name="wa",
                                bufs=2)
                wn = winvp.tile([128, tc], f32, tag="wn", name="wn", bufs=2)
                wb = winvp.tile([128, 2 * tc], f32, tag="wb", name="wb",
                                bufs=2)
                nc.vector.reciprocal_approx_fast(wa[:], wla[:])
                nc.vector.reciprocal_approx_fast(wn[:], wln[:])
                nc.vector.reciprocal_approx_fast(wb[:], wlb[:])
                # prefold winv into bf16 weights (wide where possible)
                winv_g[g] = (wa, wn, wb)
                ks = []
                for k in range(nkt):
                    bfa = wbfp.tile(
                        [128, 2 * tc], bf16, tag="bfa", name="bfa", bufs=6
                    )
                    fold_engine.tensor_tensor(
                        bfa[:], wt_f_g[g][k][:, 0 : 2 * tc], wa[:], Op.mult
                    )
                    bfn = wbfp.tile(
                        [128, tc], bf16, tag="bfn", name="bfn", bufs=6
                    )
                    fold_engine.tensor_tensor(
                        bfn[:], wt_f_g[g][k][:, 2 * tc : 3 * tc], wn[:],
                        Op.mult,
                    )
                    bfb = wbfp.tile(
                        [128, 2 * tc], bf16, tag="bfb", name="bfb", bufs=6
                    )
                    fold_engine.tensor_tensor(
                        bfb[:], wt_f_g[g][k][:, 3 * tc : 5 * tc], wb[:],
                        Op.mult,
                    )
                    ks.append((bfa, bfn, bfb))
                wbf_g[g] = ks

            def rhs_of(g, k, ci5):
                bfa, bfn, bfb = wbf_g[g][k]
                if ci5 < 2:
                    return bfa[:, ci5 * tc : (ci5 + 1) * tc]
                if ci5 == 2:
                    return bfn[:]
                return bfb[:, (ci5 - 3) * tc : (ci5 - 2) * tc]

            # ---- prologue ----
            wt_dma(0)
            for k in range(nkt):
                nc.sync.dma_start(
                    xT[:, k * b : (k + 1) * b], xt_ext[k]
                )
            x_tiles = []
            for bi in range(nb):
                x_t = xin.tile([128, d], f32, tag="x_t", name="x_t", bufs=nb)
                nc.sync.dma_start(x_t[:], x_ext[bi * 128 : (bi + 1) * 128, :])
                x_tiles.append(x_t)
            nc.sync.dma_start(labrel_t[:], labrel_ext[:])

            prep(0, nc.vector)

            wt_dma(1)
            wt_dma(2)
            ws_tiles = []
            for bi in range(nb):
                ws_t = wsin.tile(
                    [128, d], f32, tag="ws_t", name="ws_t", bufs=nb
                )
                nc.sync.dma_start(
                    ws_t[:], wsel_ext[bi * 128 : (bi + 1) * 128, :]
                )
                ws_tiles.append(ws_t)

            # flat per-block scatter offsets: p*gw device term +
            # host-encoded g*128*gw + col (OOB sentinel off-shard)
            nc.gpsimd.iota(
                iota_i[:], pattern=[[0, nb]], base=0, channel_multiplier=gw
            )
            nc.vector.tensor_copy(iota_f[:], iota_i[:])
            nc.vector.tensor_copy(rel_f[:], labrel_t[:])
            nc.vector.tensor_add(o1[:], iota_f[:], rel_f[:])
            nc.vector.tensor_copy(offs_i[:], o1[:])

            # x prep: row sumsq via ACT square+accumulate;
            # xinvS = S/||x|| feeds the copies as per-partition scale
            for bi in range(nb):
                sq = xsc.tile([128, d], f32, tag="sq", name="sq", bufs=2)
                nc.scalar.activation(
                    sq[:], x_tiles[bi][:], Act.Square,
                    accum_out=ss8[:, bi : bi + 1],
                )
            nc.scalar.activation(
                xnS8[:], ss8[:], Act.Sqrt, scale=1.0 / (S * S)
            )
            nc.vector.reciprocal(xinvS8[:], xnS8[:])
            nc.vector.tensor_scalar_mul(xinv8[:], xinvS8[:], 1.0 / S)

            prep(1, nc.gpsimd)

            # ---- main loop ----
            for g in range(ngr):
                for bi in range(nb):
                    if bi == 1:
                        if g == 0:
                            wt_dma(3)
                        elif g == 1:
                            wt_dma(4)
                        if g + 2 < ngr:
                            prep(g + 2, nc.gpsimd)
                        if g == 2:
                            # wsel row sumsq (ACT, accumulate)
                            for bj in range(nb):
                                sq = xsc.tile(
                                    [128, d], f32, tag="sq", name="sq", bufs=2
                                )
                                nc.scalar.activation(
                                    sq[:], ws_tiles[bj][:], Act.Square,
                                    accum_out=wss8[:, bj : bj + 1],
                                )
                        if g == 3:
                            # raw dot x . wsel per row (mults on gpsimd,
                            # reduce on DVE)
                            for bj in range(nb):
                                pr = xsc.tile(
                                    [128, d], f32, tag="pr", name="pr", bufs=2
                                )
                                nc.gpsimd.tensor_tensor(
                                    pr[:], x_tiles[bj][:], ws_tiles[bj][:],
                                    Op.mult,
                                )
                                nc.vector.reduce_sum(
                                    rawdot8[:, bj : bj + 1], pr[:], axis=X
                                )
                            nc.scalar.activation(wn8[:], wss8[:], Act.Sqrt)
                            nc.vector.reciprocal(wsinv8[:], wn8[:])
                    if g == ngr - 1 and bi == 0:
                        # margin math (tiny [128, nb] chain on DVE) --
                        # early so the per-block scatters can pipeline
                        # with the final bulk DMAs
                        tgt8 = persist.tile([128, nb], f32)
                        nc.vector.tensor_mul(tgt8[:], rawdot8[:], xinv8[:])
                        nc.vector.tensor_mul(tgt8[:], tgt8[:], wsinv8[:])
                        tsq = persist.tile([128, nb], f32)
                        nc.vector.tensor_mul(tsq[:], tgt8[:], tgt8[:])
                        om = persist.tile([128, nb], f32)
                        nc.vector.tensor_scalar(
                            om[:], tsq[:], -1.0, 1.0, Op.mult, Op.add
                        )
                        nc.vector.tensor_scalar_max(om[:], om[:], 0.0)
                        sine8 = persist.tile([128, nb], f32)
                        nc.scalar.activation(sine8[:], om[:], Act.Sqrt)
                        phi8 = persist.tile([128, nb], f32)
                        nc.vector.tensor_scalar_mul(phi8[:], tgt8[:], COS_M)
                        ssin8 = persist.tile([128, nb], f32)
                        nc.vector.tensor_scalar_mul(ssin8[:], sine8[:], SIN_M)
                        nc.vector.tensor_sub(phi8[:], phi8[:], ssin8[:])
                        mask8 = persist.tile([128, nb], mybir.dt.uint8)
                        nc.vector.tensor_scalar(
                            mask8[:], tgt8[:], 0.0, None, Op.is_gt
                        )
                        selv8 = persist.tile([128, nb], f32)
                        nc.vector.select(selv8[:], mask8[:], phi8[:], tgt8[:])
                        nc.vector.tensor_scalar_mul(newv8[:], selv8[:], S)

                    ops_g = [
                        po.tile([128, tc], f32, tag="ops", name="ops")
                        for _ in range(ncg)
                    ]
                    for k in range(nkt):
                        col = k * b + bi * 128
                        lhs = xT[:, col : col + 128]
                        for ci5 in range(ncg):
                            nc.tensor.matmul(
                                ops_g[ci5][:],
                                lhsT=lhs,
                                rhs=rhs_of(g, k, ci5),
                                start=(k == 0),
                                stop=(k == nkt - 1),
                            )
                    stw = stage.tile(
                        [128, gw], bf16, tag="stw", name="stw", bufs=5
                    )
                    xsc_ap = xinvS8[:, bi : bi + 1]
                    for ci5 in range(ncg):
                        dst = stw[:, ci5 * tc : (ci5 + 1) * tc]
                        if ci5 < 2 or (ci5 == 2 and bi % 4 != 3):
                            nc.scalar.mul(dst, ops_g[ci5][:], xsc_ap)
                        else:
                            nc.vector.tensor_scalar_mul(
                                dst, ops_g[ci5][:], xsc_ap
                            )
                    nc.sync.dma_start(out_blocks[bi][g], stw[:])
                    if g == ngr - 1:
                        # per-block margin scatter: depends only on this
                        # block's bulk DMAs, so it pipelines with drain
                        out_flat = out_blocks[bi][:].rearrange(
                            "g r (c one) -> (g r c) one", one=1
                        )
                        nc.gpsimd.indirect_dma_start(
                            out=out_flat,
                            out_offset=bass.IndirectOffsetOnAxis(
                                ap=offs_i[:, bi : bi + 1], axis=0
                            ),
                            in_=newv8[:, bi : bi + 1],
                            in_offset=None,
                            bounds_check=ngr * 128 * gw - 1,
                            oob_is_err=False,
                        )

    nc.compile()
    return nc


def host_prep(cfg: Cfg, input, label, weight):
    x = np.ascontiguousarray(np.asarray(input, dtype=np.float32))
    xt = np.ascontiguousarray(
        x.T.reshape(cfg.nkt, 128, cfg.b).astype(ml_dtypes.bfloat16)
    )
    w = np.asarray(weight, dtype=np.float32)
    lab = np.asarray(label).astype(np.int64)
    wsel = np.ascontiguousarray(w[lab])
    wt_all = np.ascontiguousarray(w.T)  # [D, C], relayout only
    in_maps = []
    for core in range(cfg.ncores):
        sl = slice(core * cfg.cs, (core + 1) * cfg.cs)
        wt = (
            np.ascontiguousarray(wt_all[:, sl])
            .reshape(cfg.nkt, 128, cfg.cs)
            .astype(ml_dtypes.bfloat16)
        )
        rel = lab - core * cfg.cs
        inb = (rel >= 0) & (rel < cfg.cs)
        relc = np.where(inb, rel, 0)
        g_of = relc // cfg.gw
        col_of = relc % cfg.gw
        rel2 = np.where(inb, g_of * (128 * cfg.gw) + col_of, OOB).astype(
            np.int32
        )
        labrel = np.ascontiguousarray(rel2.reshape(cfg.nb, 128).T)
        in_maps.append(
            {"x": x, "xt": xt, "wt": wt, "wsel": wsel, "labrel": labrel}
        )
    return in_maps


def run(cfg: Cfg, nc, in_maps, **kw):
    from concourse.bass_utils import run_bass_kernel_spmd

    try:
        res = run_bass_kernel_spmd(
            nc, in_maps, core_ids=list(range(cfg.ncores)), **kw
        )
    except Exception:
        # rare transient device faults have been observed; retry once
        res = run_bass_kernel_spmd(
            nc, in_maps, core_ids=list(range(cfg.ncores)), **kw
        )
    out = np.empty((cfg.b, cfg.c), dtype=np.float32)
    for c in range(cfg.ncores):
        for bi in range(cfg.nb):
            blk = res.results[c][f"out{bi}"]  # [ngr, 128, gw] bf16
            out[bi * 128 : (bi + 1) * 128, c * cfg.cs : (c + 1) * cfg.cs] = (
                blk.transpose(1, 0, 2).reshape(128, cfg.cs).astype(np.float32)
            )
    return out, res


_cache = {}


def kernel(input, label, weight):
    cfg = Cfg()
    if cfg not in _cache:
        _cache[cfg] = build(cfg)
    in_maps = host_prep(cfg, input, label, weight)
    out, _ = run(cfg, _cache[cfg], in_maps)
    return out
